# revision 15
# baseline (speedup 1.0000x reference)
"""Causal self-attention (B=2, T=2048, C=1024, H=16) on 8 trn2 NeuronCores.

Sharding: data-parallel over B (2) x tensor-parallel over head groups (4),
so each of the 8 cores handles one batch element and 4 heads end-to-end:
QKV projection (its W_attn column slice), full-T causal attention for its
4 heads, and the partial output projection (its W_proj row slice). The
host sums the 4 per-batch partials and adds biases.

Device dataflow (PE-minimal):
  x^T arrives pre-transposed (and bf16) from the host - no PE transposes.
  QKV in bf16:  Q^T/K^T ([d, t]) via W^T.T @ x^T; V ([t, d]) via x^T.T @ Wv
  S^T[k, q] = K^T.T @ Q^T per head in fp32r (causal block-skipped/trimmed)
  P = exp(S/8) on ScalarE (f32), diagonal-block mask on Pool (gpsimd)
  y^T = (V|1).T @ P^T accumulated in PSUM fp32r (rowsum rides along row 64)
  normalize: rowsum -> partition_broadcast -> reciprocal -> per-half mul;
  head-B half moved to partitions 64:128 via DVE stream_shuffle
  out^T = Wp_local.T @ y^T -> bf16 partial [1024, 2048]; host sums/biases.

Pipelined emission: QKV(tg0) runs ci-outer so PE overlaps the prologue
DMA; then stage i = attn(qi=i) + QKV(tg=i+1) + proj(tg=i) so the Tile
scheduler can fill exp-wait PE gaps with projection work.
"""

import numpy as np

import concourse.bass as bass
import concourse.mybir as mybir
import concourse.tile as tile
from concourse import bacc
from concourse.bass_utils import run_bass_kernel_spmd

F32 = mybir.dt.float32
F32R = mybir.dt.float32r
BF16 = mybir.dt.bfloat16
AF = mybir.ActivationFunctionType

B, T, C, H = 2, 2048, 1024, 16
HD = C // H          # 64
NCORES = 8
CTILES = C // 128    # 8 contraction chunks
TT = T // 128        # 16 token tiles of 128
QG = T // 512        # 4 q-groups of 512

IDENT32 = list(range(32))  # identity stream-shuffle mask


def build_nc(reps=1):
    nc = bacc.Bacc("TRN2", target_bir_lowering=False)

    xt_d = nc.declare_dram_parameter("xt_b", [C, T], BF16, isOutput=False)
    w_d = nc.declare_dram_parameter("w_l", [C, 768], BF16, isOutput=False)
    bqk_d = nc.declare_dram_parameter("b_qk", [4, 128], F32, isOutput=False)
    wp_d = nc.declare_dram_parameter("wp_l", [256, C], F32R, isOutput=False)
    out_d = nc.declare_dram_parameter("out_T", [C, T], BF16, isOutput=True)

    with tile.TileContext(nc) as tc:
        for _ in range(reps):
            with tc.tile_pool(name="persist", bufs=1) as pp:
                _build_body(nc, tc, pp, xt_d, w_d, bqk_d, wp_d, out_d)
    nc.compile()
    return nc


def _build_body(nc, tc, pp, xt_d, w_d, bqk_d, wp_d, out_d):
    # ---- constants ----
    # causal mask for S^T diagonal blocks: keep where q(col) >= k(row)
    m0 = pp.tile([128, 128], F32, tag="m0")
    nc.gpsimd.memset(m0, 1.0)
    nc.gpsimd.affine_select(out=m0, in_=m0, compare_op=mybir.AluOpType.is_ge,
                            fill=0.0, base=0, pattern=[[1, 128]], channel_multiplier=-1)

    b_sb = pp.tile([128, 4], F32, tag="b_sb")
    nc.sync.dma_start(out=b_sb, in_=bqk_d.ap().rearrange("j p -> p j"))

    # ---- persistent activations ----
    qk_pair = [pp.tile([128, T], F32R, tag=f"qkp{j}", name=f"qkp{j}") for j in range(4)]
    # each 65-col head slot is [1 | V]: the leading ones column makes the
    # softmax rowsum ride out on PSUM partition 0
    ones_c = pp.tile([128, 4], F32, tag="ones_c")
    nc.gpsimd.memset(ones_c, 1.0)
    vt = [pp.tile([128, 260], F32R, tag=f"v{t}", name=f"v{t}") for t in range(TT)]
    for t in range(TT):
        nc.vector.tensor_copy(
            vt[t].rearrange("p (h c) -> p h c", c=65)[:, :, 0:1], ones_c)
    y_un = [pp.tile([128, T], F32R, tag=f"y{hp}", name=f"y{hp}") for hp in range(2)]

    wt = [pp.tile([128, 768], BF16, tag=f"w{ci}", name=f"w{ci}") for ci in range(CTILES)]
    wp = [pp.tile([128, C], F32R, tag=f"wp{k}", name=f"wp{k}") for k in range(2)]

    def QT(h):
        lo = 64 * (h % 2)
        return qk_pair[h // 2][lo:lo + 64, :]

    def KT(h):
        lo = 64 * (h % 2)
        return qk_pair[2 + h // 2][lo:lo + 64, :]

    with tc.tile_pool(name="xt", bufs=2) as xt_pool, \
         tc.tile_pool(name="qv", bufs=2, space="PSUM") as qv_pool, \
         tc.tile_pool(name="s", bufs=2, space="PSUM") as s_pool, \
         tc.tile_pool(name="y", bufs=2, space="PSUM") as y_pool, \
         tc.tile_pool(name="pb", bufs=4) as p_pool, \
         tc.tile_pool(name="stg", bufs=2) as st_pool, \
         tc.tile_pool(name="rr", bufs=2) as rr_pool, \
         tc.tile_pool(name="osb", bufs=4) as osb_pool:

        def load_xt(tg):
            x_t = xt_pool.tile([128, 8 * 512], BF16, tag="xt", name=f"xt{tg}")
            for ci in range(CTILES):
                nc.sync.dma_start(
                    out=x_t[:, 512 * ci:512 * (ci + 1)],
                    in_=xt_d[128 * ci:128 * (ci + 1), 512 * tg:512 * (tg + 1)])
            return x_t

        def qk_drain(jc, ps, tg):
            # PSUM -> SBUF with per-partition bias add (DVE)
            nc.vector.tensor_scalar_add(
                qk_pair[jc][:, 512 * tg:512 * (tg + 1)], ps, b_sb[:, jc:jc + 1])

        def v_drain(tg, tp, pv):
            for j in range(2):
                t = 4 * tg + 2 * tp + j
                nc.vector.tensor_copy(
                    vt[t].rearrange("p (h c) -> p h c", c=65)[:, :, 1:65],
                    pv[:, 256 * j:256 * (j + 1)].rearrange("p (h c) -> p h c", c=64))

        def qkv_prologue(x_t):
            # QK ci-outer so PE chases the prologue DMA chunk-by-chunk; each
            # of the 4 accumulation series owns a distinct PSUM bank (two
            # borrowed s_pool tiles, one series per 512-col half).
            sq = [s_pool.tile([128, 1024], F32, tag="s", name=f"pqk{i}") for i in range(2)]
            for ci in range(CTILES):
                st, sp = (ci == 0), (ci == CTILES - 1)
                for jc in range(4):
                    nc.tensor.matmul(sq[jc // 2][:, 512 * (jc % 2):512 * (jc % 2 + 1)],
                                     wt[ci][:, 128 * jc:128 * (jc + 1)],
                                     x_t[:, 512 * ci:512 * (ci + 1)],
                                     start=st, stop=sp)
            for jc in range(4):
                qk_drain(jc, sq[jc // 2][:, 512 * (jc % 2):512 * (jc % 2 + 1)], 0)
            # V: sequential series per tq half (bank-sharing with interleaved
            # series corrupts PSUM accumulation)
            for tp in range(2):
                pv = qv_pool.tile([128, 512], F32, tag="qv", name=f"pv{tp}")
                for j in range(2):
                    tq = 2 * tp + j
                    for ci in range(CTILES):
                        nc.tensor.matmul(
                            pv[:, 256 * j:256 * (j + 1)],
                            x_t[:, 512 * ci + 128 * tq:512 * ci + 128 * (tq + 1)],
                            wt[ci][:, 512:768],
                            start=(ci == 0), stop=(ci == CTILES - 1))
                v_drain(0, tp, pv)

        def qkv(tg, x_t):
            for jc in range(4):
                ps = qv_pool.tile([128, 512], F32, tag="qv", name=f"qk{tg}{jc}")
                for ci in range(CTILES):
                    nc.tensor.matmul(ps, wt[ci][:, 128 * jc:128 * (jc + 1)],
                                     x_t[:, 512 * ci:512 * (ci + 1)],
                                     start=(ci == 0), stop=(ci == CTILES - 1))
                qk_drain(jc, ps, tg)
            for tp in range(2):
                pv = qv_pool.tile([128, 512], F32, tag="qv", name=f"v{tg}{tp}")
                for j in range(2):
                    tq = 2 * tp + j
                    for ci in range(CTILES):
                        nc.tensor.matmul(
                            pv[:, 256 * j:256 * (j + 1)],
                            x_t[:, 512 * ci + 128 * tq:512 * ci + 128 * (tq + 1)],
                            wt[ci][:, 512:768],
                            start=(ci == 0), stop=(ci == CTILES - 1))
                v_drain(tg, tp, pv)

        def attn_group(hp, qi):
            hA, hB = 2 * hp, 2 * hp + 1
            psA = y_pool.tile([128, 512], F32, tag="y", name=f"yA{hp}{qi}")
            psB = y_pool.tile([128, 512], F32, tag="y", name=f"yB{hp}{qi}")
            nkt = 4 * qi + 4
            for ki in range(nkt):
                r = ki - 4 * qi
                soff = 0 if r < 1 else (128 * r if r < 3 else 256)
                sAB = s_pool.tile([128, 1024], F32, tag="s")
                for half, h in ((0, hA), (1, hB)):
                    nc.tensor.matmul(
                        sAB[:, 512 * half + soff:512 * half + 512],
                        KT(h)[:, 128 * ki:128 * (ki + 1)],
                        QT(h)[:, 512 * qi + soff:512 * (qi + 1)],
                        start=True, stop=True)
                pAB = p_pool.tile([128, 1024], F32R, tag="p")
                if r >= 1:
                    we = 128 * r
                    nc.scalar.activation(
                        pAB.rearrange("p (h q) -> p h q", h=2)[:, :, we:512],
                        sAB.rearrange("p (h q) -> p h q", h=2)[:, :, we:512],
                        AF.Exp, scale=1.0 / np.sqrt(HD))
                    if r == 3:
                        # fp32r PV reads [256:512]; zero the never-exp'd cols
                        nc.vector.memset(pAB[:, 256:384].bitcast(F32), 0.0)
                        nc.vector.memset(pAB[:, 768:896].bitcast(F32), 0.0)
                else:
                    nc.scalar.activation(pAB, sAB, AF.Exp, scale=1.0 / np.sqrt(HD))
                if r >= 0:
                    for half in range(2):
                        base = 512 * half + 128 * r
                        nc.vector.tensor_mul(pAB[:, base:base + 128],
                                             pAB[:, base:base + 128], m0)
                woff = 0 if r < 1 else (128 * r if r < 3 else 256)
                st, sp = (ki == 0), (ki == nkt - 1)
                nc.tensor.matmul(psA[0:65, woff:512],
                                 vt[ki][:, 65 * hA:65 * hA + 65],
                                 pAB[:, woff:512],
                                 start=st, stop=sp)
                nc.tensor.matmul(psB[0:65, woff:512],
                                 vt[ki][:, 65 * hB:65 * hB + 65],
                                 pAB[:, 512 + woff:1024],
                                 start=st, stop=sp)

            # ---- drain + softmax normalization ----
            # ps[0] = rowsum (ones ride-along), ps[1:65] = y^T rows
            qsl = slice(512 * qi, 512 * (qi + 1))
            for ps, lo, tag in ((psA, 0, "rbA"), (psB, 64, "rbB")):
                st = st_pool.tile([128, 512], F32R, tag="st")
                nc.vector.tensor_copy(st[0:65, :], ps[0:65, :])
                rb = rr_pool.tile([128, 512], F32R, tag=tag)
                nc.gpsimd.partition_broadcast(rb, st[0:1, :], channels=128)
                with nc.allow_low_precision(reason="softmax denom reciprocal"):
                    nc.vector.reciprocal(rb[lo:lo + 64, :], rb[lo:lo + 64, :])
                nc.sync.dma_start(out=y_un[hp][lo:lo + 64, qsl], in_=st[1:65, :])
                nc.vector.tensor_mul(y_un[hp][lo:lo + 64, qsl],
                                     y_un[hp][lo:lo + 64, qsl], rb[lo:lo + 64, :])

        def proj(tg):
            for co in range(CTILES):
                pr = qv_pool.tile([128, 512], F32, tag="qv", name=f"pr{tg}{co}")
                for k in range(2):
                    nc.tensor.matmul(pr, wp[k][:, 128 * co:128 * (co + 1)],
                                     y_un[k][:, 512 * tg:512 * (tg + 1)],
                                     start=(k == 0), stop=(k == 1))
                osb = osb_pool.tile([128, 512], BF16, tag="osb")
                if co % 2 == 0:
                    nc.scalar.copy(osb, pr)
                else:
                    nc.vector.tensor_copy(osb, pr)
                nc.sync.dma_start(
                    out=out_d[128 * co:128 * (co + 1), 512 * tg:512 * (tg + 1)],
                    in_=osb)

        # ================= pipelined emission =================
        # prologue: weights + first x block; QKV(0) chases the DMA
        for ci in range(CTILES):
            nc.sync.dma_start(out=wt[ci], in_=w_d[128 * ci:128 * (ci + 1), :])
        x_cur = load_xt(0)
        for k in range(2):
            nc.sync.dma_start(out=wp[k], in_=wp_d[128 * k:128 * (k + 1), :])
        qkv_prologue(x_cur)
        x_nxt = load_xt(1)

        for qi in range(QG):
            for hp in range(2):
                attn_group(hp, qi)
            if qi + 1 < QG:
                x_cur = x_nxt
                qkv(qi + 1, x_cur)
                if qi + 2 < QG:
                    x_nxt = load_xt(qi + 2)
            proj(qi)


_NC = None


def _get_nc():
    global _NC
    if _NC is None:
        _NC = build_nc()
    return _NC


def kernel(x, W_attn, b_attn, W_proj, b_proj, _trace=False):
    import ml_dtypes

    x = np.asarray(x, dtype=np.float32)
    W_attn = np.asarray(W_attn, dtype=np.float32)
    b_attn = np.asarray(b_attn, dtype=np.float32)
    W_proj = np.asarray(W_proj, dtype=np.float32)
    b_proj = np.asarray(b_proj, dtype=np.float32)
    bf16 = ml_dtypes.bfloat16

    xt_b = [np.ascontiguousarray(x[b].T).astype(bf16) for b in range(B)]

    in_maps = []
    for core in range(NCORES):
        b, hg = divmod(core, 4)
        qs = [W_attn[:, 64 * (4 * hg + h):64 * (4 * hg + h + 1)] for h in range(4)]
        ks = [W_attn[:, C + 64 * (4 * hg + h):C + 64 * (4 * hg + h + 1)] for h in range(4)]
        vs = [W_attn[:, 2 * C + 64 * (4 * hg + h):2 * C + 64 * (4 * hg + h + 1)] for h in range(4)]
        w_l = np.concatenate(qs + ks + vs, axis=1)
        bq = [b_attn[64 * (4 * hg + h):64 * (4 * hg + h + 1)] for h in range(4)]
        bk = [b_attn[C + 64 * (4 * hg + h):C + 64 * (4 * hg + h + 1)] for h in range(4)]
        b_qk = np.stack([np.concatenate(bq[0:2]), np.concatenate(bq[2:4]),
                         np.concatenate(bk[0:2]), np.concatenate(bk[2:4])])
        wp_l = np.concatenate(
            [W_proj[64 * (4 * hg + h):64 * (4 * hg + h + 1), :] for h in range(4)], axis=0)
        in_maps.append({
            "xt_b": xt_b[b],
            "w_l": np.ascontiguousarray(w_l).astype(bf16),
            "b_qk": np.ascontiguousarray(b_qk, dtype=np.float32),
            "wp_l": np.ascontiguousarray(wp_l, dtype=np.float32),
        })

    nc = _get_nc()
    kwargs = {}
    if _trace:
        kwargs = dict(trace=True, trace_cores=[0])
    res = run_bass_kernel_spmd(nc, in_maps, core_ids=list(range(NCORES)), **kwargs)

    # V-bias folds into the output bias because softmax rows sum to 1.
    bias_total = b_proj + b_attn[2 * C:3 * C] @ W_proj
    out = np.empty((B, T, C), dtype=np.float32)
    for b in range(B):
        acc = res.results[4 * b]["out_T"].astype(np.float32)
        for hg in range(1, 4):
            acc = acc + res.results[4 * b + hg]["out_T"].astype(np.float32)
        out[b] = acc.T + bias_total[None, :]
    if _trace:
        return out, res
    return out


# revision 17
# speedup vs baseline: 1.0249x; 1.0249x over previous
"""Causal self-attention (B=2, T=2048, C=1024, H=16) on 8 trn2 NeuronCores.

Sharding: data-parallel over B (2) x tensor-parallel over head groups (4),
so each of the 8 cores handles one batch element and 4 heads end-to-end:
QKV projection (its W_attn column slice), full-T causal attention for its
4 heads, and the partial output projection (its W_proj row slice). The
host sums the 4 per-batch partials and adds biases.

Device dataflow (PE-minimal):
  x^T arrives pre-transposed (and bf16) from the host - no PE transposes.
  QKV in bf16:  Q^T/K^T ([d, t]) via W^T.T @ x^T; V ([t, d]) via x^T.T @ Wv
  S^T[k, q] = K^T.T @ Q^T per head in fp32r (causal block-skipped/trimmed)
  P = exp(S/8) on ScalarE (f32), diagonal-block mask on Pool (gpsimd)
  y^T = (V|1).T @ P^T accumulated in PSUM fp32r (rowsum rides along row 64)
  normalize: rowsum -> partition_broadcast -> reciprocal -> per-half mul;
  head-B half moved to partitions 64:128 via DVE stream_shuffle
  out^T = Wp_local.T @ y^T -> bf16 partial [1024, 2048]; host sums/biases.

Pipelined emission: QKV(tg0) runs ci-outer so PE overlaps the prologue
DMA; then stage i = attn(qi=i) + QKV(tg=i+1) + proj(tg=i) so the Tile
scheduler can fill exp-wait PE gaps with projection work.
"""

import numpy as np

import concourse.bass as bass
import concourse.mybir as mybir
import concourse.tile as tile
from concourse import bacc
from concourse.bass_utils import run_bass_kernel_spmd

F32 = mybir.dt.float32
F32R = mybir.dt.float32r
BF16 = mybir.dt.bfloat16
AF = mybir.ActivationFunctionType

B, T, C, H = 2, 2048, 1024, 16
HD = C // H          # 64
NCORES = 8
CTILES = C // 128    # 8 contraction chunks
TT = T // 128        # 16 token tiles of 128
QG = T // 512        # 4 q-groups of 512

IDENT32 = list(range(32))  # identity stream-shuffle mask


def build_nc(reps=1):
    nc = bacc.Bacc("TRN2", target_bir_lowering=False)

    xt_d = nc.declare_dram_parameter("xt_b", [C, T], BF16, isOutput=False)
    w_d = nc.declare_dram_parameter("w_l", [C, 768], BF16, isOutput=False)
    bqk_d = nc.declare_dram_parameter("b_qk", [4, 128], F32, isOutput=False)
    wp_d = nc.declare_dram_parameter("wp_l", [256, C], F32R, isOutput=False)
    out_d = nc.declare_dram_parameter("out_T", [C, T], BF16, isOutput=True)

    with tile.TileContext(nc) as tc:
        for _ in range(reps):
            with tc.tile_pool(name="persist", bufs=1) as pp:
                _build_body(nc, tc, pp, xt_d, w_d, bqk_d, wp_d, out_d)
    nc.compile()
    return nc


def _build_body(nc, tc, pp, xt_d, w_d, bqk_d, wp_d, out_d):
    # ---- constants ----
    # causal mask for S^T diagonal blocks: keep where q(col) >= k(row)
    m0 = pp.tile([128, 128], F32, tag="m0")
    nc.gpsimd.memset(m0, 1.0)
    nc.gpsimd.affine_select(out=m0, in_=m0, compare_op=mybir.AluOpType.is_ge,
                            fill=0.0, base=0, pattern=[[1, 128]], channel_multiplier=-1)

    b_sb = pp.tile([128, 4], F32, tag="b_sb")
    nc.sync.dma_start(out=b_sb, in_=bqk_d.ap().rearrange("j p -> p j"))

    # ---- persistent activations ----
    qk_pair = [pp.tile([128, T], F32R, tag=f"qkp{j}", name=f"qkp{j}") for j in range(4)]
    # each 65-col head slot is [1 | V]: the leading ones column makes the
    # softmax rowsum ride out on PSUM partition 0
    ones_c = pp.tile([128, 4], F32, tag="ones_c")
    nc.gpsimd.memset(ones_c, 1.0)
    vt = [pp.tile([128, 260], F32R, tag=f"v{t}", name=f"v{t}") for t in range(TT)]
    for t in range(TT):
        nc.vector.tensor_copy(
            vt[t].rearrange("p (h c) -> p h c", c=65)[:, :, 0:1], ones_c)
    y_un = [pp.tile([128, T], F32R, tag=f"y{hp}", name=f"y{hp}") for hp in range(2)]

    wt = [pp.tile([128, 768], BF16, tag=f"w{ci}", name=f"w{ci}") for ci in range(CTILES)]
    wp = [pp.tile([128, C], F32R, tag=f"wp{k}", name=f"wp{k}") for k in range(2)]

    def QT(h):
        lo = 64 * (h % 2)
        return qk_pair[h // 2][lo:lo + 64, :]

    def KT(h):
        lo = 64 * (h % 2)
        return qk_pair[2 + h // 2][lo:lo + 64, :]

    with tc.tile_pool(name="xt", bufs=2) as xt_pool, \
         tc.tile_pool(name="qv", bufs=2, space="PSUM") as qv_pool, \
         tc.tile_pool(name="s", bufs=2, space="PSUM") as s_pool, \
         tc.tile_pool(name="y", bufs=2, space="PSUM") as y_pool, \
         tc.tile_pool(name="pb", bufs=6) as p_pool, \
         tc.tile_pool(name="stg", bufs=2) as st_pool, \
         tc.tile_pool(name="rr", bufs=2) as rr_pool, \
         tc.tile_pool(name="osb", bufs=4) as osb_pool:

        def load_xt(tg):
            x_t = xt_pool.tile([128, 8 * 512], BF16, tag="xt", name=f"xt{tg}")
            for ci in range(CTILES):
                nc.sync.dma_start(
                    out=x_t[:, 512 * ci:512 * (ci + 1)],
                    in_=xt_d[128 * ci:128 * (ci + 1), 512 * tg:512 * (tg + 1)])
            return x_t

        def qk_drain(jc, ps, tg):
            # PSUM -> SBUF with per-partition bias add (DVE)
            nc.vector.tensor_scalar_add(
                qk_pair[jc][:, 512 * tg:512 * (tg + 1)], ps, b_sb[:, jc:jc + 1])

        def v_drain(tg, tp, pv):
            for j in range(2):
                t = 4 * tg + 2 * tp + j
                nc.vector.tensor_copy(
                    vt[t].rearrange("p (h c) -> p h c", c=65)[:, :, 1:65],
                    pv[:, 256 * j:256 * (j + 1)].rearrange("p (h c) -> p h c", c=64))

        def qkv_prologue(x_t):
            # QK ci-outer so PE chases the prologue DMA chunk-by-chunk; each
            # of the 4 accumulation series owns a distinct PSUM bank (two
            # borrowed s_pool tiles, one series per 512-col half).
            sq = [s_pool.tile([128, 1024], F32, tag="s", name=f"pqk{i}") for i in range(2)]
            for ci in range(CTILES):
                st, sp = (ci == 0), (ci == CTILES - 1)
                for jc in range(4):
                    nc.tensor.matmul(sq[jc // 2][:, 512 * (jc % 2):512 * (jc % 2 + 1)],
                                     wt[ci][:, 128 * jc:128 * (jc + 1)],
                                     x_t[:, 512 * ci:512 * (ci + 1)],
                                     start=st, stop=sp)
            for jc in range(4):
                qk_drain(jc, sq[jc // 2][:, 512 * (jc % 2):512 * (jc % 2 + 1)], 0)
            # V: sequential series per tq half (bank-sharing with interleaved
            # series corrupts PSUM accumulation)
            for tp in range(2):
                pv = qv_pool.tile([128, 512], F32, tag="qv", name=f"pv{tp}")
                for j in range(2):
                    tq = 2 * tp + j
                    for ci in range(CTILES):
                        nc.tensor.matmul(
                            pv[:, 256 * j:256 * (j + 1)],
                            x_t[:, 512 * ci + 128 * tq:512 * ci + 128 * (tq + 1)],
                            wt[ci][:, 512:768],
                            start=(ci == 0), stop=(ci == CTILES - 1))
                v_drain(0, tp, pv)

        def qkv(tg, x_t):
            for jc in range(4):
                ps = qv_pool.tile([128, 512], F32, tag="qv", name=f"qk{tg}{jc}")
                for ci in range(CTILES):
                    nc.tensor.matmul(ps, wt[ci][:, 128 * jc:128 * (jc + 1)],
                                     x_t[:, 512 * ci:512 * (ci + 1)],
                                     start=(ci == 0), stop=(ci == CTILES - 1))
                qk_drain(jc, ps, tg)
            for tp in range(2):
                pv = qv_pool.tile([128, 512], F32, tag="qv", name=f"v{tg}{tp}")
                for j in range(2):
                    tq = 2 * tp + j
                    for ci in range(CTILES):
                        nc.tensor.matmul(
                            pv[:, 256 * j:256 * (j + 1)],
                            x_t[:, 512 * ci + 128 * tq:512 * ci + 128 * (tq + 1)],
                            wt[ci][:, 512:768],
                            start=(ci == 0), stop=(ci == CTILES - 1))
                v_drain(tg, tp, pv)

        def attn_group(hp, qi):
            hA, hB = 2 * hp, 2 * hp + 1
            psA = y_pool.tile([128, 512], F32, tag="y", name=f"yA{hp}{qi}")
            psB = y_pool.tile([128, 512], F32, tag="y", name=f"yB{hp}{qi}")
            nkt = 4 * qi + 4
            for ki in range(nkt):
                r = ki - 4 * qi
                soff = 0 if r < 1 else (128 * r if r < 3 else 256)
                sAB = s_pool.tile([128, 1024], F32, tag="s")
                for half, h in ((0, hA), (1, hB)):
                    nc.tensor.matmul(
                        sAB[:, 512 * half + soff:512 * half + 512],
                        KT(h)[:, 128 * ki:128 * (ki + 1)],
                        QT(h)[:, 512 * qi + soff:512 * (qi + 1)],
                        start=True, stop=True)
                pAB = p_pool.tile([128, 1024], F32R, tag="p")
                if r >= 1:
                    we = 128 * r
                    nc.scalar.activation(
                        pAB.rearrange("p (h q) -> p h q", h=2)[:, :, we:512],
                        sAB.rearrange("p (h q) -> p h q", h=2)[:, :, we:512],
                        AF.Exp, scale=1.0 / np.sqrt(HD))
                    if r == 3:
                        # fp32r PV reads [256:512]; zero the never-exp'd cols
                        nc.vector.memset(pAB[:, 256:384].bitcast(F32), 0.0)
                        nc.vector.memset(pAB[:, 768:896].bitcast(F32), 0.0)
                else:
                    nc.scalar.activation(pAB, sAB, AF.Exp, scale=1.0 / np.sqrt(HD))
                if r >= 0:
                    for half in range(2):
                        base = 512 * half + 128 * r
                        nc.vector.tensor_mul(pAB[:, base:base + 128],
                                             pAB[:, base:base + 128], m0)
                woff = 0 if r < 1 else (128 * r if r < 3 else 256)
                st, sp = (ki == 0), (ki == nkt - 1)
                nc.tensor.matmul(psA[0:65, woff:512],
                                 vt[ki][:, 65 * hA:65 * hA + 65],
                                 pAB[:, woff:512],
                                 start=st, stop=sp)
                nc.tensor.matmul(psB[0:65, woff:512],
                                 vt[ki][:, 65 * hB:65 * hB + 65],
                                 pAB[:, 512 + woff:1024],
                                 start=st, stop=sp)

            # ---- drain + softmax normalization ----
            # ps[0] = rowsum (ones ride-along), ps[1:65] = y^T rows
            qsl = slice(512 * qi, 512 * (qi + 1))
            for ps, lo, tag in ((psA, 0, "rbA"), (psB, 64, "rbB")):
                st = st_pool.tile([128, 512], F32R, tag="st")
                nc.vector.tensor_copy(st[0:65, :], ps[0:65, :])
                rb = rr_pool.tile([128, 512], F32R, tag=tag)
                nc.gpsimd.partition_broadcast(rb, st[0:1, :], channels=128)
                with nc.allow_low_precision(reason="softmax denom reciprocal"):
                    nc.vector.reciprocal(rb[lo:lo + 64, :], rb[lo:lo + 64, :])
                nc.sync.dma_start(out=y_un[hp][lo:lo + 64, qsl], in_=st[1:65, :])
                nc.vector.tensor_mul(y_un[hp][lo:lo + 64, qsl],
                                     y_un[hp][lo:lo + 64, qsl], rb[lo:lo + 64, :])

        def proj(tg):
            for co in range(CTILES):
                pr = qv_pool.tile([128, 512], F32, tag="qv", name=f"pr{tg}{co}")
                for k in range(2):
                    nc.tensor.matmul(pr, wp[k][:, 128 * co:128 * (co + 1)],
                                     y_un[k][:, 512 * tg:512 * (tg + 1)],
                                     start=(k == 0), stop=(k == 1))
                osb = osb_pool.tile([128, 512], BF16, tag="osb")
                if co % 2 == 0:
                    nc.scalar.copy(osb, pr)
                else:
                    nc.vector.tensor_copy(osb, pr)
                nc.sync.dma_start(
                    out=out_d[128 * co:128 * (co + 1), 512 * tg:512 * (tg + 1)],
                    in_=osb)

        # ================= pipelined emission =================
        # prologue: weights + first x block; QKV(0) chases the DMA
        for ci in range(CTILES):
            nc.sync.dma_start(out=wt[ci], in_=w_d[128 * ci:128 * (ci + 1), :])
        x_cur = load_xt(0)
        for k in range(2):
            nc.sync.dma_start(out=wp[k], in_=wp_d[128 * k:128 * (k + 1), :])
        qkv(0, x_cur)
        x_nxt = load_xt(1)

        for qi in range(QG):
            for hp in range(2):
                attn_group(hp, qi)
            if qi + 1 < QG:
                x_cur = x_nxt
                qkv(qi + 1, x_cur)
                if qi + 2 < QG:
                    x_nxt = load_xt(qi + 2)
            proj(qi)


_NC = None


def _get_nc():
    global _NC
    if _NC is None:
        _NC = build_nc()
    return _NC


def kernel(x, W_attn, b_attn, W_proj, b_proj, _trace=False):
    import ml_dtypes

    x = np.asarray(x, dtype=np.float32)
    W_attn = np.asarray(W_attn, dtype=np.float32)
    b_attn = np.asarray(b_attn, dtype=np.float32)
    W_proj = np.asarray(W_proj, dtype=np.float32)
    b_proj = np.asarray(b_proj, dtype=np.float32)
    bf16 = ml_dtypes.bfloat16

    xt_b = [np.ascontiguousarray(x[b].T).astype(bf16) for b in range(B)]

    in_maps = []
    for core in range(NCORES):
        b, hg = divmod(core, 4)
        qs = [W_attn[:, 64 * (4 * hg + h):64 * (4 * hg + h + 1)] for h in range(4)]
        ks = [W_attn[:, C + 64 * (4 * hg + h):C + 64 * (4 * hg + h + 1)] for h in range(4)]
        vs = [W_attn[:, 2 * C + 64 * (4 * hg + h):2 * C + 64 * (4 * hg + h + 1)] for h in range(4)]
        w_l = np.concatenate(qs + ks + vs, axis=1)
        bq = [b_attn[64 * (4 * hg + h):64 * (4 * hg + h + 1)] for h in range(4)]
        bk = [b_attn[C + 64 * (4 * hg + h):C + 64 * (4 * hg + h + 1)] for h in range(4)]
        b_qk = np.stack([np.concatenate(bq[0:2]), np.concatenate(bq[2:4]),
                         np.concatenate(bk[0:2]), np.concatenate(bk[2:4])])
        wp_l = np.concatenate(
            [W_proj[64 * (4 * hg + h):64 * (4 * hg + h + 1), :] for h in range(4)], axis=0)
        in_maps.append({
            "xt_b": xt_b[b],
            "w_l": np.ascontiguousarray(w_l).astype(bf16),
            "b_qk": np.ascontiguousarray(b_qk, dtype=np.float32),
            "wp_l": np.ascontiguousarray(wp_l, dtype=np.float32),
        })

    nc = _get_nc()
    kwargs = {}
    if _trace:
        kwargs = dict(trace=True, trace_cores=[0])
    res = run_bass_kernel_spmd(nc, in_maps, core_ids=list(range(NCORES)), **kwargs)

    # V-bias folds into the output bias because softmax rows sum to 1.
    bias_total = b_proj + b_attn[2 * C:3 * C] @ W_proj
    out = np.empty((B, T, C), dtype=np.float32)
    for b in range(B):
        acc = res.results[4 * b]["out_T"].astype(np.float32)
        for hg in range(1, 4):
            acc = acc + res.results[4 * b + hg]["out_T"].astype(np.float32)
        out[b] = acc.T + bias_total[None, :]
    if _trace:
        return out, res
    return out
